# revision 32
# baseline (speedup 1.0000x reference)
"""FConv2d via 9-tap matmul convolution on 8 TRN2 NeuronCores.

The reference computes ifft3(fft3(x) * fft3(W)) over a (128, 65, 65) grid,
crops, channel-subsamples by 4 and reshapes.  That is exactly:

  out[b, s*8+n, u, v] = sum_{dc<32, di<3, dj<3}
      W[n, dc, di, dj] * x_zp[b, (4s-dc) mod 128, u+1-di, v+1-dj]

(x_zp = x zero-padded by 1 spatially; the channel axis wraps circularly).
Per 3x3 tap this is a [256 x 128] channel-mixing matmul against a spatially
shifted view of x.  The tap matrices A are a pure scatter of W (no
arithmetic), built on host.  Sharding: data-parallel over batch, one
element per core.

Two kernel modes:

* dense_f32r: 9 taps x 2 co-halves of [128x128]@[128x512] float32r matmuls
  (fp32 storage, 1 cyc/col).  The tap matrix is 75% zeros.

* pack8_fp16 (default): exploits the block-banded structure.  Each 32-wide
  co-block m only reads a 44-channel window; with x stored twice (identity
  and channels rotated by +31 partitions) every window aligns inside a
  64-partition block, so each tap is 8 concurrent 64x32 PE tiles (full
  array, zero wasted columns) -> half the PE column streams of dense.
  fp16 operands (f32r forbids column tiling), fp32 PSUM accumulate.
"""

import numpy as np

import concourse.bass as bass
import concourse.tile as tile
from concourse import bacc, mybir
from concourse.bass_utils import run_bass_kernel_spmd

L = 64
CIN = 128
COUT = 256
NF = 8        # num filters
KS = 3        # kernel size
NTAP = KS * KS
B = 8
N_CORES = 8

MODE = "pack4_fp16"          # or "pack8_fp16" / "dense_f32r"
PACK16 = MODE.startswith("pack")   # harness compat: selects packed A build

ROT = 31                     # channel rotation of the second x copy
NXCHUNK = 4                  # x DMA chunks (rows per chunk = L / NXCHUNK)
XROWS = L // NXCHUNK
HALF = NTAP * 128            # dense-A columns per output-channel half


def _window_rot(m: int) -> bool:
    """True if co-block m's channel window needs the rotated x copy."""
    return (m % 4) < 2


def _afull(W: np.ndarray) -> np.ndarray:
    """Dense tap tensor Afull[c, t, co] (f64 precision scatter of W)."""
    c = np.arange(CIN)
    Afull = np.zeros((CIN, NTAP, COUT), np.float32)
    for co in range(COUT):
        s_, n = co // NF, co % NF
        dc = (4 * s_ - c) % CIN
        mask = dc < 32
        for e in range(KS):
            for f in range(KS):
                Afull[mask, e * KS + f, co] = W[n, dc[mask], 2 - e, 2 - f]
    return Afull


def _build_A(W: np.ndarray) -> np.ndarray:
    """Dense layout [128, 2*9*128] f32: A[c, h*1152 + t*128 + m]."""
    Afull = _afull(W)
    A = np.zeros((CIN, 2, NTAP, 128), np.float32)
    for h in range(2):
        A[:, h] = Afull[:, :, h * 128:(h + 1) * 128]
    return np.ascontiguousarray(A.reshape(CIN, 2 * HALF))


def _build_A_pack(W: np.ndarray) -> np.ndarray:
    """Packed fp16 layout [128, 9*128] for the 8-tile 64x32 scheme.

    Partitions [64*(m//4), +64), cols [t*128 + (m%4)*32, +32) hold co-block
    m's [64c x 32co] coupling for tap t, with the channel->partition map
    p = (c + 31) % 128 for m%4 < 2 (rotated x copy) and p = c otherwise.
    """
    Afull = _afull(W)
    P = np.zeros((CIN, NTAP, 128), np.float32)
    p = np.arange(CIN)
    c_rot = (p - ROT) % CIN          # channel held at partition p, rotated
    for m in range(8):
        kb, s = m // 4, m % 4
        rows = slice(64 * kb, 64 * kb + 64)
        chans = c_rot[rows] if _window_rot(m) else p[rows]
        P[rows, :, s * 32:s * 32 + 32] = Afull[chans, :, 32 * m:32 * m + 32]
    assert np.abs(P).sum() == np.abs(Afull).sum(), "block cover is leaky"
    return np.ascontiguousarray(P.reshape(CIN, NTAP * 128)).astype(np.float16)


def _build_A_pack4(W: np.ndarray) -> np.ndarray:
    """Packed fp16 layout [128, 9*128] for the 4-tile 64x64 scheme.

    Tile kp covers co [64*kp, +64); row half kb = kp//2; kp even uses the
    rotated x copy (p = (c+31)%128), kp odd the identity copy.  Block at
    partitions [64*kb, +64), cols [t*128 + 64*(kp%2), +64).
    """
    Afull = _afull(W)
    P = np.zeros((CIN, NTAP, 128), np.float32)
    p = np.arange(CIN)
    c_rot = (p - ROT) % CIN
    for kp in range(4):
        kb = kp // 2
        rows = slice(64 * kb, 64 * kb + 64)
        chans = c_rot[rows] if kp % 2 == 0 else p[rows]
        P[rows, :, 64 * (kp % 2):64 * (kp % 2) + 64] = \
            Afull[chans, :, 64 * kp:64 * kp + 64]
    assert np.abs(P).sum() == np.abs(Afull).sum(), "block cover is leaky"
    return np.ascontiguousarray(P.reshape(CIN, NTAP * 128)).astype(np.float16)


def _dedup_ldweights(nc):
    """Remove InstLdweights that reload the exact weights already resident
    in the same PE tile slot.  Tile lowering expands every matmul into
    Ldweights + Matmult(ldweights=False); with q-inner loops the 3 trailing
    reloads per (tap, slot) are redundant.  Any waits/updates on a removed
    load are migrated to the next PE instruction (its paired matmult),
    which executes no earlier than the load would have.
    """
    PE = mybir.EngineType.PE
    for blk in nc.main_func.blocks:
        resident = {}
        pending_sync = []
        keep = []
        for inst in blk.instructions:
            if getattr(inst, "engine", None) != PE:
                keep.append(inst)
                continue
            if isinstance(inst, mybir.InstLdweights):
                pos = tuple(inst.tile_position or (0, 0))
                ap = inst.ins[0]
                sig = (ap.memref, ap.offset, str(ap.ap), str(ap.dtype),
                       str(inst.tile_size))
                if resident.get(pos) == sig:
                    if inst.sync_info is not None:
                        pending_sync.append(inst.sync_info)
                    continue
                resident[pos] = sig
            elif isinstance(inst, mybir.InstMatmult):
                if pending_sync:
                    si = inst.sync_info
                    if si is None:
                        si = mybir.SyncInfo(on_wait=[], on_update=[])
                        inst.sync_info = si
                    for ps in pending_sync:
                        si.on_wait.extend(ps.on_wait)
                        si.on_update.extend(ps.on_update)
                    pending_sync = []
            else:
                # unknown PE instruction: be conservative, weights unknown
                resident.clear()
            keep.append(inst)
        assert not pending_sync, "dangling sync from removed ldweights"
        blk.instructions[:] = keep


def _build_program_pack8():
    nc = bacc.Bacc("TRN2", target_bir_lowering=False, debug=False,
                   num_devices=N_CORES)
    F16 = mybir.dt.float16
    x_ap = nc.dram_tensor("x", [CIN, L, L], F16,
                          kind="ExternalInput").ap()
    xr_ap = nc.dram_tensor("xr", [CIN, L, L], F16,
                           kind="ExternalInput").ap()
    a_ap = nc.dram_tensor("A", [CIN, NTAP * 128], F16,
                          kind="ExternalInput").ap()
    out_ap = nc.dram_tensor("out", [COUT, L, L], mybir.dt.float32,
                            kind="ExternalOutput").ap()

    with tile.TileContext(nc) as tc:
        with (
            tc.tile_pool(name="const", bufs=1) as const_pool,
            tc.tile_pool(name="psum", bufs=8, space="PSUM") as psum_pool,
            tc.tile_pool(name="outs", bufs=8) as out_pool,
        ):
            # --- PE warmup -----------------------------------------------
            # Dummy matmuls during the input-DMA window push the HAM
            # activity monitor to K=8/8 before the real stream starts
            # (otherwise the first pass runs at 1.2 GHz).  Results land in
            # a scratch PSUM bank and are never read.
            wz = const_pool.tile([128, 512], F16)
            nc.vector.memset(wz[:], 0.0)
            pswa = psum_pool.tile([128, 512], mybir.dt.float32,
                                  name="ps_warm_a", tag="psbank")
            pswb = psum_pool.tile([128, 512], mybir.dt.float32,
                                  name="ps_warm_b", tag="psbank")
            # 4 concurrent 64x64 tiles per round: full-array activity (the
            # HAM busy metric needs it) in the same tiling mode as the real
            # stream (a mode switch would cost a drain)
            for _ in range(11):
                for psd, rp, cp in ((pswa, 0, 0), (pswa, 64, 64),
                                    (pswb, 64, 0), (pswb, 0, 64)):
                    nc.tensor.matmul(psd[cp:cp + 64, :],
                                     wz[rp:rp + 64, 0:64], wz[rp:rp + 64, :],
                                     start=True, stop=True,
                                     tile_position=(rp, cp),
                                     skip_group_check=True)

            # --- input staging -------------------------------------------
            A_sb = const_pool.tile([CIN, NTAP * 128], F16)
            nc.scalar.dma_start(A_sb[:], a_ap[:])

            # xp: zero-padded fp16 x; xpr: same for the host-rotated copy
            # (partition p holds channel (p - 31) % 128).
            xp = const_pool.tile([CIN, L + 2, L + 2], F16)
            xpr = const_pool.tile([CIN, L + 2, L + 2], F16)
            for t_ in (xp, xpr):
                nc.vector.memset(t_[:, 0, :], 0.0)
                nc.vector.memset(t_[:, L + 1, :], 0.0)
                nc.vector.memset(t_[:, :, 0], 0.0)
                nc.vector.memset(t_[:, :, L + 1], 0.0)
            # DMA into contiguous staging (2KB/partition bursts), then
            # DVE-copy into the padded layout; the two x copies ride the
            # two HWDGE rings in parallel.
            xs = const_pool.tile([CIN, L, L], F16)
            xrs = const_pool.tile([CIN, L, L], F16)
            NCH, CR = 8, L // 8
            for k in range(NCH):
                rows_x = slice(CR * k, CR * (k + 1))
                rows_p = slice(1 + CR * k, 1 + CR * (k + 1))
                nc.sync.dma_start(xs[:, rows_x, :], x_ap[:, rows_x, :])
                nc.scalar.dma_start(xrs[:, rows_x, :], xr_ap[:, rows_x, :])
                nc.vector.tensor_copy(xp[:, rows_p, 1:L + 1],
                                      xs[:, rows_x, :])
                nc.vector.tensor_copy(xpr[:, rows_p, 1:L + 1],
                                       xrs[:, rows_x, :])

            # --- packed 9-tap matmul conv --------------------------------
            # Two passes of 4 spatial chunks; per (tap, slot) one explicit
            # LDWEIGHTS feeds 4 non-self-loading matmuls (weight reuse).
            ROWS = 8
            NQ = L // ROWS
            # pass pattern: big first pass, small later passes so the
            # final drain tail is short
            passes = [[0, 1], [2, 3], [4, 5], [6, 7]]
            for qs in passes:
                banks = {}
                for q in qs:
                    for h in range(2):
                        banks[(q, h)] = psum_pool.tile(
                            [128, ROWS * L], mybir.dt.float32,
                            name=f"psbank_{q}_{h}", tag="psbank")
                for t in range(NTAP):
                    e, f = t // KS, t % KS
                    if MODE == "pack4_fp16":
                        # (kp, row half, col pos, width, bank h, uses rot x)
                        tiles = [(kp, kp // 2, 64 * (kp % 2), 64, kp // 2,
                                  kp % 2 == 0) for kp in (1, 3, 0, 2)]
                    else:
                        tiles = [(m, m // 4, 32 * (m % 4), 32, m // 4,
                                  _window_rot(m)) for m in range(8)]
                    for _, kb, cpos, cw, h, use_rot in tiles:
                        src = xpr if use_rot else xp
                        lhsT = A_sb[64 * kb:64 * kb + 64,
                                    t * 128 + cpos:t * 128 + cpos + cw]
                        for q in qs:
                            bank = banks[(q, h)]
                            rhs = src[64 * kb:64 * kb + 64,
                                      ROWS * q + e:ROWS * q + e + ROWS,
                                      f:f + L]
                            nc.tensor.matmul(
                                bank[cpos:cpos + cw, :], lhsT, rhs,
                                start=(t == 0), stop=(t == NTAP - 1),
                                tile_position=(64 * kb, cpos),
                                skip_group_check=True)
                for q in qs:
                    for h in range(2):
                        o = out_pool.tile([128, ROWS * L], mybir.dt.float32)
                        nc.vector.tensor_copy(o[:], banks[(q, h)][:])
                        nc.sync.dma_start(
                            out_ap[h * 128:h * 128 + 128,
                                   ROWS * q:ROWS * q + ROWS, :],
                            o[:].rearrange("p (a b) -> p a b", a=ROWS))
    _dedup_ldweights(nc)
    nc.compile()
    return nc


def _build_program_dense():
    nc = bacc.Bacc("TRN2", target_bir_lowering=False, debug=False,
                   num_devices=N_CORES)
    x_ap = nc.dram_tensor("x", [CIN, L, L], mybir.dt.float32,
                          kind="ExternalInput").ap()
    a_ap = nc.dram_tensor("A", [CIN, 2 * HALF], mybir.dt.float32,
                          kind="ExternalInput").ap()
    out_ap = nc.dram_tensor("out", [COUT, L, L], mybir.dt.float32,
                            kind="ExternalOutput").ap()
    MM_DT = mybir.dt.float32r

    with tile.TileContext(nc) as tc:
        with (
            tc.tile_pool(name="const", bufs=1) as const_pool,
            tc.tile_pool(name="psum", bufs=4, space="PSUM") as psum_pool,
            tc.tile_pool(name="outs", bufs=4) as out_pool,
        ):
            xs = const_pool.tile([CIN, L, L], mybir.dt.float32)
            for k in range(NXCHUNK):
                nc.sync.dma_start(xs[:, XROWS * k:XROWS * (k + 1), :],
                                  x_ap[:, XROWS * k:XROWS * (k + 1), :])

            A_raw = const_pool.tile([CIN, 2 * HALF], mybir.dt.float32)
            A_sb = const_pool.tile([CIN, 2 * HALF], MM_DT)
            for h in range(2):
                nc.scalar.dma_start(A_raw[:, h * HALF:(h + 1) * HALF],
                                    a_ap[:, h * HALF:(h + 1) * HALF])
                nc.vector.tensor_copy(A_sb[:, h * HALF:(h + 1) * HALF],
                                      A_raw[:, h * HALF:(h + 1) * HALF])

            zrow = const_pool.tile([CIN, L + 2], mybir.dt.float32)
            nc.vector.memset(zrow[:], 0.0)
            xp = const_pool.tile([CIN, L + 2, L + 2], MM_DT)
            nc.vector.tensor_copy(xp[:, 0, :], zrow[:])
            nc.vector.tensor_copy(xp[:, L + 1, :], zrow[:])
            nc.vector.tensor_copy(xp[:, :, 0], zrow[:])
            nc.vector.tensor_copy(xp[:, :, L + 1], zrow[:])
            for k in range(NXCHUNK):
                nc.vector.tensor_copy(
                    xp[:, 1 + XROWS * k:1 + XROWS * (k + 1), 1:L + 1],
                    xs[:, XROWS * k:XROWS * (k + 1), :])

            ROWS = 8
            NQ = L // ROWS
            for h in range(2):
                for q in range(NQ):
                    ps = psum_pool.tile([128, ROWS * L], mybir.dt.float32)
                    for t in range(NTAP):
                        e, f = t // KS, t % KS
                        lhsT = A_sb[:, h * HALF + t * 128:
                                    h * HALF + t * 128 + 128]
                        rhs = xp[:, ROWS * q + e:ROWS * q + e + ROWS,
                                 f:f + L]
                        nc.tensor.matmul(ps[:], lhsT, rhs,
                                         start=(t == 0), stop=(t == NTAP - 1))
                    o = out_pool.tile([128, ROWS * L], mybir.dt.float32)
                    nc.vector.tensor_copy(o[:], ps[:])
                    nc.sync.dma_start(
                        out_ap[h * 128:h * 128 + 128,
                               ROWS * q:ROWS * q + ROWS, :],
                        o[:].rearrange("p (a b) -> p a b", a=ROWS))
    nc.compile()
    return nc


def _build_program():
    if MODE.startswith("pack"):
        return _build_program_pack8()
    return _build_program_dense()


_PROGRAM = None


def _get_program():
    global _PROGRAM
    if _PROGRAM is None:
        _PROGRAM = _build_program()
    return _PROGRAM


def kernel(x: np.ndarray, W: np.ndarray) -> np.ndarray:
    x = np.ascontiguousarray(np.asarray(x, dtype=np.float32))
    W = np.asarray(W, dtype=np.float32)
    if MODE.startswith("pack"):
        A = _build_A_pack4(W) if MODE == "pack4_fp16" else _build_A_pack(W)
        perm = (np.arange(CIN) - ROT) % CIN   # xr[p] = x[(p-31)%128]
        xh = x.astype(np.float16)
        in_maps = [{"x": np.ascontiguousarray(xh[b]),
                    "xr": np.ascontiguousarray(xh[b][perm]),
                    "A": A} for b in range(B)]
    else:
        A = _build_A(W)
        in_maps = [{"x": np.ascontiguousarray(x[b]), "A": A}
                   for b in range(B)]
    nc = _get_program()
    res = run_bass_kernel_spmd(nc, in_maps, list(range(N_CORES)))
    return np.stack([res.results[i]["out"] for i in range(N_CORES)], axis=0)
